# revision 6
# baseline (speedup 1.0000x reference)
"""GQA attention kernel for 8 Trainium2 NeuronCores.

Sharding: core = (batch b, kv_group g), b in {0,1}, g in {0..3}.
Each core computes the 4 heads of one KV group for one batch and the
partial output projection for those heads; the host sums the 4 group
partials per batch.  Zero duplicated compute across cores.

Per-core layout choices (all matmuls run in float32r = full PE rate):
  - host passes xT = x[b].T so every projection has contraction on
    partitions without any on-device transpose of x
  - QT/KT are produced directly in [head_dim, S] layout; V in natural
    [S, head_dim] layout (via a PE transpose of VT)
  - scoresT[t, q] = KT_tile^T @ QT  -> exp on ACT (no max subtraction:
    scores are ~N(0,1) after folding 1/sqrt(D) into Wq, exp is safe)
  - softmax denominators via an all-ones stationary matmul (partition
    reduction on PE); the redundant 128 identical rows make the
    reciprocal + normalize plain full-tile DVE ops (no broadcasts)
  - attention output is accumulated transposed (outT[d, q]) so the
    output projection needs no transpose either; the host transposes
    the final [E, S] partial back to [S, E].
"""

import numpy as np

# problem shape (hardcoded per contract)
B, S, E = 2, 2048, 2048
H, G, D = 16, 4, 128
R = H // G          # heads per kv group = 4
KV = G * D          # 512
ST = S // 128       # 16 t-tiles
ET = E // 128       # 16 e-tiles
SC = S // 512       # 4 s-chunks
NPAIR = S // 1024   # 2 q-chunk pairs

_cache = {}


def _split_multi_waits(nc, maxw=1):
    """Walrus in this container accepts only one sync-wait per
    instruction; move extra waits onto preceding same-engine NoOps."""
    from concourse import mybir

    n_split = 0
    for fn in nc.m.functions:
        for bb in fn.blocks:
            out = []
            changed = False
            for inst in bb.instructions:
                si = inst.sync_info
                waits = list(si.on_wait or []) if si is not None else []
                if len(waits) > maxw:
                    changed = True
                    n_split += 1
                    head, tail = waits[:-maxw], waits[-maxw:]
                    for j in range(0, len(head), maxw):
                        nop = mybir.InstNoOp(
                            name=f"{inst.name}-wsplit{j}", ins=[], outs=[]
                        )
                        nop.engine = inst.engine
                        nop.sync_info = mybir.SyncInfo(
                            on_wait=head[j : j + maxw], on_update=[]
                        )
                        out.append(nop)
                    si.on_wait = tail
                out.append(inst)
            if changed:
                bb.instructions = out
    return n_split


def _build_program():
    import concourse.bass as bass
    import concourse.tile as tile
    from concourse import mybir
    from concourse.masks import make_identity

    F32R = mybir.dt.float32r
    F32 = mybir.dt.float32
    Exp = mybir.ActivationFunctionType.Exp
    Mult = mybir.AluOpType.mult

    nc = bass.Bass(target_bir_lowering=False)

    xT = nc.dram_tensor("xT", [E, S], F32R, kind="ExternalInput")
    wq = nc.dram_tensor("wq", [E, R * D], F32R, kind="ExternalInput")
    wk = nc.dram_tensor("wk", [E, D], F32R, kind="ExternalInput")
    wv = nc.dram_tensor("wv", [E, D], F32R, kind="ExternalInput")
    wo = nc.dram_tensor("wo", [R * D, E], F32R, kind="ExternalInput")
    bqv = nc.dram_tensor("bqv", [R * D], F32, kind="ExternalInput")
    bkv = nc.dram_tensor("bkv", [D], F32, kind="ExternalInput")
    bvv = nc.dram_tensor("bvv", [D], F32, kind="ExternalInput")
    otd = nc.dram_tensor("ot", [E, S], F32, kind="ExternalOutput")

    with tile.TileContext(nc) as tc:
        import contextlib

        with contextlib.ExitStack() as ctx:
            consts = ctx.enter_context(tc.tile_pool(name="consts", bufs=1))
            qkvt = ctx.enter_context(tc.tile_pool(name="qkvt", bufs=1))

            ident_f = consts.tile([128, 128], F32)
            make_identity(nc, ident_f)
            ident = consts.tile([128, 128], F32R)
            nc.vector.tensor_copy(ident, ident_f)
            ones_f = consts.tile([128, 128], F32)
            nc.gpsimd.memset(ones_f, 1.0)
            ones = consts.tile([128, 128], F32R)
            nc.vector.tensor_copy(ones, ones_f)
            bq_sb = consts.tile([128, R], F32)
            nc.sync.dma_start(bq_sb, bqv.rearrange("(o p) -> p o", p=128))
            bk_sb = consts.tile([128, 1], F32)
            nc.sync.dma_start(bk_sb, bkv.rearrange("(o p) -> p o", p=128))
            bv_sb = consts.tile([128, 1], F32)
            nc.sync.dma_start(bv_sb, bvv.rearrange("(o p) -> p o", p=128))

            QT = qkvt.tile([128, R, S], F32R)    # QT[d, h, s]
            KT = qkvt.tile([128, S], F32R)       # KT[d, t]
            V = qkvt.tile([128, ST, D], F32R)    # V[t%128, tt, d]

            # ---- phase 1: QKV^T projections + V transpose ----
            with tc.tile_pool(name="wts", bufs=1) as wpool, \
                 tc.tile_pool(name="xts", bufs=2) as xtpool, \
                 tc.tile_pool(name="vt", bufs=1) as vtpool, \
                 tc.tile_pool(name="ps1", bufs=3, space="PSUM") as ps1, \
                 tc.tile_pool(name="psv", bufs=2, space="PSUM") as psv:
                wq_sb = wpool.tile([128, ET, R * D], F32R)
                nc.sync.dma_start(wq_sb, wq.rearrange("(o p) m -> p o m", p=128))
                wk_sb = wpool.tile([128, ET, D], F32R)
                nc.sync.dma_start(wk_sb, wk.rearrange("(o p) m -> p o m", p=128))
                wv_sb = wpool.tile([128, ET, D], F32R)
                nc.sync.dma_start(wv_sb, wv.rearrange("(o p) m -> p o m", p=128))
                VT = vtpool.tile([128, S], F32R)

                for sc in range(SC):
                    xtile = xtpool.tile([128, ET, 512], F32R, tag="xt")
                    for e in range(ET):
                        nc.sync.dma_start(
                            xtile[:, e],
                            xT[e * 128 : (e + 1) * 128, sc * 512 : (sc + 1) * 512],
                        )
                    cs = slice(sc * 512, (sc + 1) * 512)
                    for ot in range(R + 2):
                        psum = ps1.tile([128, 512], F32, tag="p1")
                        for e in range(ET):
                            if ot < R:
                                lhsT = wq_sb[:, e, ot * 128 : (ot + 1) * 128]
                            elif ot == R:
                                lhsT = wk_sb[:, e]
                            else:
                                lhsT = wv_sb[:, e]
                            nc.tensor.matmul(
                                psum, lhsT, xtile[:, e],
                                start=(e == 0), stop=(e == ET - 1),
                            )
                        if ot < R:
                            nc.scalar.add(QT[:, ot, cs], psum, bq_sb[:, ot : ot + 1])
                        elif ot == R:
                            nc.scalar.add(KT[:, cs], psum, bk_sb[:, 0:1])
                        else:
                            nc.scalar.add(VT[:, cs], psum, bv_sb[:, 0:1])

                for tt in range(ST):
                    ps = psv.tile([128, 128], F32R, tag="pv")
                    nc.tensor.transpose(ps, VT[:, tt * 128 : (tt + 1) * 128], ident)
                    nc.vector.tensor_copy(V[:, tt], ps)

            # ---- phase 2: attention per head ----
            p23 = ctx.enter_context(tc.tile_pool(name="p23", bufs=1))
            outT = p23.tile([128, R, S], F32R)  # normalized attn outT[d, h, s]
            wo_sb = p23.tile([128, R, E], F32R)
            nc.sync.dma_start(wo_sb, wo.rearrange("(o p) m -> p o m", p=128))
            with tc.tile_pool(name="probs", bufs=3) as probs_pool, \
                 tc.tile_pool(name="recip", bufs=2) as rpool, \
                 tc.tile_pool(name="ps_s", bufs=2, space="PSUM") as ps_s, \
                 tc.tile_pool(name="ps_sum", bufs=1, space="PSUM") as ps_sum, \
                 tc.tile_pool(name="ps_av", bufs=1, space="PSUM") as ps_av:

                for h in range(R):
                    for pr in range(NPAIR):
                        q0 = pr * 1024
                        sums_ps = ps_sum.tile([128, 1024], F32, tag="sums")
                        out_ps = ps_av.tile([128, 1024], F32, tag="av")
                        for tt in range(ST):
                            pss = ps_s.tile([128, 1024], F32, tag="scores")
                            kslice = KT[:, tt * 128 : (tt + 1) * 128]
                            for hf in range(2):
                                nc.tensor.matmul(
                                    pss[:, hf * 512 : (hf + 1) * 512],
                                    kslice,
                                    QT[:, h, q0 + hf * 512 : q0 + (hf + 1) * 512],
                                    start=True, stop=True,
                                )
                            pt = probs_pool.tile([128, 1024], F32R, tag="probs")
                            nc.scalar.activation(pt, pss, Exp)
                            for hf in range(2):
                                hs = slice(hf * 512, (hf + 1) * 512)
                                nc.tensor.matmul(
                                    sums_ps[:, hs], ones, pt[:, hs],
                                    start=(tt == 0), stop=(tt == ST - 1),
                                )
                                nc.tensor.matmul(
                                    out_ps[:, hs], V[:, tt], pt[:, hs],
                                    start=(tt == 0), stop=(tt == ST - 1),
                                )
                        rc = rpool.tile([128, 1024], F32, tag="recip")
                        nc.vector.reciprocal(rc, sums_ps)
                        nc.vector.tensor_tensor(
                            outT[:, h, q0 : q0 + 1024], out_ps, rc, Mult
                        )

            # ---- phase 3: output projection (transposed) ----
            with tc.tile_pool(name="ostage", bufs=3) as ostage, \
                 tc.tile_pool(name="ps_o", bufs=4, space="PSUM") as ps_o:
                for et in range(ET):
                    for sc in range(SC):
                        ps = ps_o.tile([128, 512], F32, tag="po")
                        for h in range(R):
                            nc.tensor.matmul(
                                ps,
                                wo_sb[:, h, et * 128 : (et + 1) * 128],
                                outT[:, h, sc * 512 : (sc + 1) * 512],
                                start=(h == 0), stop=(h == R - 1),
                            )
                        st = ostage.tile([128, 512], F32, tag="ost")
                        nc.vector.tensor_copy(st, ps)
                        nc.sync.dma_start(
                            otd[et * 128 : (et + 1) * 128,
                                sc * 512 : (sc + 1) * 512],
                            st,
                        )

    _split_multi_waits(nc)
    return nc


def _prepare(x, Wq, bq, Wk, bk, Wv, bv, Wo, bo):
    """Host-side sharding: build per-core input maps."""
    x = np.asarray(x, dtype=np.float32)
    Wq = np.asarray(Wq, dtype=np.float32)
    bq = np.asarray(bq, dtype=np.float32)
    Wk = np.asarray(Wk, dtype=np.float32)
    bk = np.asarray(bk, dtype=np.float32)
    Wv = np.asarray(Wv, dtype=np.float32)
    bv = np.asarray(bv, dtype=np.float32)
    Wo = np.asarray(Wo, dtype=np.float32)

    isd = np.float32(1.0 / np.sqrt(D))
    xTs = [np.ascontiguousarray(x[b].T) for b in range(B)]
    in_maps = []
    for core in range(8):
        b, g = divmod(core, G)
        in_maps.append({
            "xT": xTs[b],
            "wq": np.ascontiguousarray(Wq[:, g * R * D : (g + 1) * R * D]) * isd,
            "wk": np.ascontiguousarray(Wk[:, g * D : (g + 1) * D]),
            "wv": np.ascontiguousarray(Wv[:, g * D : (g + 1) * D]),
            "wo": np.ascontiguousarray(Wo[g * R * D : (g + 1) * R * D, :]),
            "bqv": bq[g * R * D : (g + 1) * R * D] * isd,
            "bkv": bk[g * D : (g + 1) * D],
            "bvv": bv[g * D : (g + 1) * D],
        })
    return in_maps


def _gather(results, bo):
    bo = np.asarray(bo, dtype=np.float32)
    out = np.empty((B, S, E), dtype=np.float32)
    for b in range(B):
        acc = results[b * G]["ot"].copy()
        for g in range(1, G):
            acc += results[b * G + g]["ot"]
        out[b] = acc.T + bo
    return out


def kernel(x, Wq, bq, Wk, bk, Wv, bv, Wo, bo):
    from concourse.bass_utils import run_bass_kernel_spmd

    if "nc" not in _cache:
        _cache["nc"] = _build_program()
    nc = _cache["nc"]
    in_maps = _prepare(x, Wq, bq, Wk, bk, Wv, bv, Wo, bo)
    res = run_bass_kernel_spmd(nc, in_maps, core_ids=list(range(8)))
    return _gather(res.results, bo)
